# revision 1
# baseline (speedup 1.0000x reference)
"""CollisionLoss kernel for 8 Trainium2 NeuronCores.

Data-parallel over batch: 512 trajectories split 64-per-core; each core
computes partial penalty sums, the host combines in float64 and divides by
B*N.

Per core, batches run 8 at a time (one per 16-partition group). Each batch's
200x200 ESDF map is held fp32 in SBUF as M row-decimated tables: group
partition 16g+m holds rows {m, m+M, ...} (H/M rows, F = (H/M)*W elements,
sized under the IndirectCopy ucode's 8K-element window). Rows ix and ix+1 of
a bilinear stencil live in tables (ix mod M) and ((ix+1) mod M) at the same
local index k = (ix div M)*W + iy, except ix mod M == M-1 which needs k+W in
table 0 -- so TWO GPSIMD indirect_copy calls with shared index streams k and
min(k+W, F-2) (inner size 2 fetches the iy/iy+1 pair) deliver all four
corners for every point. Which gathered stream is "row ix" vs "row ix+1" is
selected per point with is_equal masks on q = ix mod M.

Point prep (clip, floor, fracs, weights, in-range mask) runs on DVE/ACT over
all 128 partitions. Gathered pairs are redistributed from the M table
partitions back to the wrapped 128-partition layout by on-chip DMAs.
penalty = relu(3-10v)^2 and its reduction run on ACT (activation accum_out).
Out-of-range points get zeroed weights (v=0 -> penalty 9) plus a 160*(1-z)
correction -> 169 total, matching dists=-1 in the reference.

This walrus build rejects instructions carrying >1 sync wait; a post-pass
moves extra waits onto same-engine NoOps.
"""
import sys

sys.path.insert(0, "/opt/trn_rl_repo")

import numpy as np

import bass_rust
import concourse.bass as bass
import concourse.mybir as mybir
from concourse.tile import TileContext
from concourse import bass_utils
from concourse.bass_utils import run_bass_kernel_spmd

B, N, H, W = 512, 8192, 200, 200
NCORES = 8
BPC = B // NCORES            # 64 batches per core
GPB = 8                      # batches per round
ROUNDS = BPC // GPB          # 8
SW = N // 16                 # 512 point-slots per partition per batch
M = 10                       # row-decimation ways
RPT = H // M                 # rows per table (40)
F = RPT * W                  # 8000 elements per table
GK = 512                     # indices per indirect_copy call
HP = 2                       # sigma-buffer chunks per round
HN = N // HP                 # 4096 points per chunk
NSLOT = 2 * ROUNDS

F32 = mybir.dt.float32
U16 = mybir.dt.uint16
AF = mybir.ActivationFunctionType
OP = mybir.AluOpType

PROFILE = False
LAST_EXEC_NS = None

bass_utils.upload_artifacts = lambda tmpdir: "file://" + str(tmpdir)


def _split_multiwaits(nc):
    for fn in nc.m.functions:
        for bb in fn.blocks:
            insts = bb.instructions
            out = []
            changed = False
            for inst in insts:
                si = inst.sync_info
                waits = list(si.on_wait) if si is not None else []
                if len(waits) > 1:
                    for k, w in enumerate(waits[:-1]):
                        nop = bass_rust.InstNoOp(
                            name=f"{inst.name}-w{k}", engine=inst.engine)
                        nop.sync_info = bass_rust.SyncInfo(
                            on_wait=[w], on_update=[])
                        out.append(nop)
                    inst.sync_info = bass_rust.SyncInfo(
                        on_wait=[waits[-1]], on_update=list(si.on_update))
                    changed = True
                out.append(inst)
            if changed:
                bb.instructions = out


def _build():
    nc = bass.Bass()
    for v in (3.0,):
        t = nc.alloc_sbuf_tensor(f"const-float32-{v}", [128, 1], F32)
        nc.gpsimd.memset(t.ap(), v)
        nc.const_aps.aps[(F32, v)] = t.ap()
    nc.all_engine_barrier()
    ops_in = nc.dram_tensor("opState", [BPC, N * 2], F32, kind="ExternalInput")
    env_in = nc.dram_tensor("envs", [BPC, H * W], F32, kind="ExternalInput")
    part_out = nc.dram_tensor("partials", [128, NSLOT], F32,
                              kind="ExternalOutput")

    OS_t = nc.alloc_sbuf_tensor("osig", [128, N, 2], F32)
    G1P = [nc.alloc_sbuf_tensor(f"g1p{m}", [128, SW, 2], F32).ap()
           for m in range(M)]
    G2P = nc.alloc_sbuf_tensor("g2p", [128, SW, 2], F32).ap()

    with TileContext(nc) as tc:
        with (
            tc.tile_pool(name="tbl", bufs=1) as tblp,
            tc.tile_pool(name="pts", bufs=2) as ptsp,
            tc.tile_pool(name="prep", bufs=1) as prp,
            tc.tile_pool(name="widx", bufs=2) as wip,
            tc.tile_pool(name="gout", bufs=1) as gop,
            tc.tile_pool(name="comb", bufs=1) as cbp,
            tc.tile_pool(name="acc", bufs=1) as accp,
        ):
            table = tblp.tile([128, F], F32)
            t3d = table[:].rearrange("p (n d) -> p n d", d=2)  # [128,4000,2]
            partials = accp.tile([128, NSLOT], F32)

            env_ap = env_in.ap()
            ops_ap = ops_in.ap()

            for r in range(ROUNDS):
                b0 = r * GPB
                src = env_ap[b0:b0 + GPB, :].rearrange(
                    "g (h c) -> g h c", c=W)
                for m in range(M):
                    nc.sync.dma_start(out=table[m:128:16, :],
                                      in_=src[:, m:H:M, :])

                P = ptsp.tile([128, 2 * SW], F32)
                nc.sync.dma_start(
                    out=P[:],
                    in_=ops_ap[b0:b0 + GPB, :].rearrange(
                        "g (p f) -> (g p) f", p=16))

                # ---- prep: T = clip(x*10,-99,99)+99 = scaled coord - 0.5
                T = prp.tile([128, 2 * SW], F32)
                nc.vector.tensor_scalar(out=T[:], in0=P[:], scalar1=10.0,
                                        scalar2=-99.0, op0=OP.mult, op1=OP.max)
                nc.vector.tensor_scalar(out=T[:], in0=T[:], scalar1=99.0,
                                        scalar2=99.0, op0=OP.min, op1=OP.add)
                IC = prp.tile([128, 2 * SW], U16)    # floor of scaled coord
                nc.scalar.activation(out=IC[:], in_=T[:], func=AF.Copy)
                ITF = prp.tile([128, 2 * SW], F32)
                nc.scalar.activation(out=ITF[:], in_=IC[:], func=AF.Copy)
                ITFv = ITF[:].rearrange("p (s t) -> p s t", t=2)
                ITFX, ITFY = ITFv[:, :, 0], ITFv[:, :, 1]
                DXF = prp.tile([128, 2 * SW], F32)   # frac - 0.5
                nc.vector.tensor_tensor(out=DXF[:], in0=T[:], in1=ITF[:],
                                        op=OP.subtract)
                DXFv = DXF[:].rearrange("p (s t) -> p s t", t=2)
                DXFX, DXFY = DXFv[:, :, 0], DXFv[:, :, 1]

                # h = ix // M via round(ix/M - (M-1)/(2M)); q = ix - M*h
                HC = prp.tile([128, SW], U16)
                nc.scalar.activation(out=HC[:], in_=ITFX, func=AF.Copy,
                                     scale=1.0 / M, bias=-(M - 1) / (2.0 * M))
                HF = prp.tile([128, SW], F32)
                nc.scalar.activation(out=HF[:], in_=HC[:], func=AF.Copy)
                Q = prp.tile([128, SW], F32)
                nc.vector.tensor_scalar(out=Q[:], in0=HF[:],
                                        scalar1=-float(M), scalar2=None,
                                        op0=OP.mult)
                nc.vector.tensor_tensor(out=Q[:], in0=Q[:], in1=ITFX,
                                        op=OP.add)
                KF = prp.tile([128, SW], F32)        # k = h*W + iy
                nc.vector.tensor_scalar(out=KF[:], in0=HF[:],
                                        scalar1=float(W), scalar2=None,
                                        op0=OP.mult)
                nc.vector.tensor_tensor(out=KF[:], in0=KF[:], in1=ITFY,
                                        op=OP.add)
                IA = wip.tile([128, SW], U16)
                nc.scalar.activation(out=IA[:], in_=KF[:], func=AF.Copy)
                IB = wip.tile([128, SW], U16)
                nc.vector.tensor_scalar(out=IB[:], in0=KF[:],
                                        scalar1=float(W),
                                        scalar2=float(F - 2), op0=OP.add,
                                        op1=OP.min)

                # in-range mask z
                AB = prp.tile([128, 2 * SW], F32)
                nc.scalar.activation(out=AB[:], in_=P[:], func=AF.Abs)
                ABv = AB[:].rearrange("p (s t) -> p s t", t=2)
                ZM = prp.tile([128, SW], F32)
                nc.vector.tensor_tensor(out=ZM[:], in0=ABv[:, :, 0],
                                        in1=ABv[:, :, 1], op=OP.max)
                Z = prp.tile([128, SW], F32)
                nc.vector.tensor_scalar(out=Z[:], in0=ZM[:], scalar1=9.9,
                                        scalar2=None, op0=OP.is_le)
                CD = prp.tile([128, SW], F32)
                nc.scalar.activation(
                    out=CD[:], in_=Z[:], func=AF.Copy, scale=-160.0,
                    bias=160.0,
                    accum_out=partials[:, ROUNDS + r:ROUNDS + r + 1])

                # x/y fracs and row weights ax0 = (1-dx)z, ax1 = dx*z
                DXE = prp.tile([128, SW], F32)
                nc.scalar.activation(out=DXE[:], in_=DXFX, func=AF.Copy,
                                     bias=0.5)
                DYE = prp.tile([128, SW], F32)
                nc.scalar.activation(out=DYE[:], in_=DXFY, func=AF.Copy,
                                     bias=0.5)
                DYC = prp.tile([128, SW], F32)
                nc.scalar.activation(out=DYC[:], in_=DXFY, func=AF.Copy,
                                     scale=-1.0, bias=0.5)
                AX1 = prp.tile([128, SW], F32)
                nc.vector.tensor_tensor(out=AX1[:], in0=DXE[:], in1=Z[:],
                                        op=OP.mult)
                AX0 = prp.tile([128, SW], F32)
                nc.vector.tensor_tensor(out=AX0[:], in0=Z[:], in1=AX1[:],
                                        op=OP.subtract)
                # y-weight planes: A0E = ax0*(1-dy), A0O = ax0*dy, ...
                A0E = prp.tile([128, SW], F32)
                nc.vector.tensor_tensor(out=A0E[:], in0=AX0[:], in1=DYC[:],
                                        op=OP.mult)
                A0O = prp.tile([128, SW], F32)
                nc.vector.tensor_tensor(out=A0O[:], in0=AX0[:], in1=DYE[:],
                                        op=OP.mult)
                A1E = prp.tile([128, SW], F32)
                nc.vector.tensor_tensor(out=A1E[:], in0=AX1[:], in1=DYC[:],
                                        op=OP.mult)
                A1O = prp.tile([128, SW], F32)
                nc.vector.tensor_tensor(out=A1O[:], in0=AX1[:], in1=DYE[:],
                                        op=OP.mult)

                # ---- gather + redistribute (half-round sigma buffers)
                O1 = OS_t.ap()
                for c in range(N // GK):
                    i0, i1 = c * (GK // 16), (c + 1) * (GK // 16)
                    nc.gpsimd.indirect_copy(
                        O1[:, c * GK:(c + 1) * GK, :], t3d,
                        IA[:, i0:i1], True)
                for m in range(M):
                    for qq in range(16):
                        nc.sync.dma_start(
                            out=G1P[m][qq:qq + 113:16, :, :],
                            in_=O1[m:m + 113:16, :, :].rearrange(
                                "g (s u) c -> g s (u c)",
                                u=16)[:, :, 2 * qq:2 * qq + 2])
                O2 = OS_t.ap()
                for c in range(N // GK):
                    i0, i1 = c * (GK // 16), (c + 1) * (GK // 16)
                    nc.gpsimd.indirect_copy(
                        O2[:, c * GK:(c + 1) * GK, :], t3d,
                        IB[:, i0:i1], True)
                for qq in range(16):
                    nc.sync.dma_start(
                        out=G2P[qq:qq + 113:16, :, :],
                        in_=O2[0:113:16, :, :].rearrange(
                            "g (s u) c -> g s (u c)",
                            u=16)[:, :, 2 * qq:2 * qq + 2])

                # ---- select rows by q and combine
                # R0* = sum_m [q==m]*G1*[m]            (row ix planes)
                # R1* = sum_{m<M-1} [q==m]*G1*[m+1] + [q==M-1]*G2*
                R0E = cbp.tile([128, SW], F32, tag="r0e")
                R0O = cbp.tile([128, SW], F32, tag="r0o")
                R1E = cbp.tile([128, SW], F32, tag="r1e")
                R1O = cbp.tile([128, SW], F32, tag="r1o")
                TMP = cbp.tile([128, SW], F32, tag="tmp")
                for m in range(M):
                    EQ = cbp.tile([128, SW], F32, tag="eq")
                    nc.vector.tensor_scalar(out=EQ[:], in0=Q[:],
                                            scalar1=float(m), scalar2=None,
                                            op0=OP.is_equal)
                    se = (G1P[m + 1] if m < M - 1 else G2P)[:, :, 0]
                    so = (G1P[m + 1] if m < M - 1 else G2P)[:, :, 1]
                    for dst, g in ((R0E, G1P[m][:, :, 0]),
                                   (R0O, G1P[m][:, :, 1]),
                                   (R1E, se), (R1O, so)):
                        if m == 0:
                            nc.vector.tensor_tensor(out=dst[:], in0=EQ[:],
                                                    in1=g, op=OP.mult)
                        else:
                            nc.vector.tensor_tensor(out=TMP[:], in0=EQ[:],
                                                    in1=g, op=OP.mult)
                            nc.vector.tensor_tensor(out=dst[:], in0=dst[:],
                                                    in1=TMP[:], op=OP.add)

                U = cbp.tile([128, SW], F32, tag="u")
                nc.vector.tensor_tensor(out=U[:], in0=R0E[:], in1=A0E[:],
                                        op=OP.mult)
                UT = cbp.tile([128, SW], F32, tag="ut")
                for g, a in ((R0O, A0O), (R1E, A1E), (R1O, A1O)):
                    nc.vector.tensor_tensor(out=UT[:], in0=g[:], in1=a[:],
                                            op=OP.mult)
                    nc.vector.tensor_tensor(out=U[:], in0=U[:], in1=UT[:],
                                            op=OP.add)
                PS = U
                RT = cbp.tile([128, SW], F32, tag="rt")
                nc.scalar.activation(out=RT[:], in_=PS[:], func=AF.Relu,
                                     scale=-10.0, bias=3.0)
                SQ = cbp.tile([128, SW], F32, tag="sq")
                nc.scalar.activation(out=SQ[:], in_=RT[:], func=AF.Square,
                                     accum_out=partials[:, r:r + 1])

            nc.sync.dma_start(out=part_out.ap()[:], in_=partials[:])

    _split_multiwaits(nc)
    return nc


_CACHE = {}


def kernel(opState, envs):
    global LAST_EXEC_NS
    if "nc" not in _CACHE:
        _CACHE["nc"] = _build()
    nc = _CACHE["nc"]

    opState = np.ascontiguousarray(opState, dtype=np.float32)
    envs = np.ascontiguousarray(envs, dtype=np.float32)
    envs2 = envs.reshape(B, H * W)
    ops2 = opState.reshape(B, N * 2)

    in_maps = []
    for c in range(NCORES):
        sl = slice(c * BPC, (c + 1) * BPC)
        in_maps.append({
            "opState": np.ascontiguousarray(ops2[sl]),
            "envs": np.ascontiguousarray(envs2[sl]),
        })

    res = run_bass_kernel_spmd(nc, in_maps, core_ids=list(range(NCORES)),
                               trace=PROFILE)
    if res.exec_time_ns is not None:
        LAST_EXEC_NS = res.exec_time_ns

    total = np.float64(0.0)
    for r in res.results:
        total += r["partials"].astype(np.float64).sum()
    return np.float32(total / (B * N))


if __name__ == "__main__":
    d = np.load("/root/problem/work/ref_cache.npz")
    out = kernel(d["opState"], d["envs"])
    exp = float(d["out"])
    print("actual:", float(out), "expected:", exp,
          "rel:", abs(float(out) - exp) / abs(exp))



# revision 3
# speedup vs baseline: 1.1334x; 1.1334x over previous
"""CollisionLoss kernel for 8 Trainium2 NeuronCores.

Data-parallel over batch: 512 trajectories split 64-per-core; each core
computes partial penalty sums, the host combines in float64 and divides by
B*N.

Per core, batches run 8 at a time (one per 16-partition group). Each batch's
200x200 ESDF map is held fp32 in SBUF as M row-decimated tables: group
partition 16g+m holds rows {m, m+M, ...} (H/M rows, F = (H/M)*W elements,
sized under the IndirectCopy ucode's 8K-element window). Rows ix and ix+1 of
a bilinear stencil live in tables (ix mod M) and ((ix+1) mod M) at the same
local index k = (ix div M)*W + iy, except ix mod M == M-1 which needs k+W in
table 0 -- so TWO GPSIMD indirect_copy calls with shared index streams k and
min(k+W, F-2) (inner size 2 fetches the iy/iy+1 pair) deliver all four
corners for every point. Which gathered stream is "row ix" vs "row ix+1" is
selected per point with is_equal masks on q = ix mod M.

Point prep (clip, floor, fracs, weights, in-range mask) runs on DVE/ACT over
all 128 partitions. Gathered pairs are redistributed from the M table
partitions back to the wrapped 128-partition layout by on-chip DMAs.
penalty = relu(3-10v)^2 and its reduction run on ACT (activation accum_out).
Out-of-range points get zeroed weights (v=0 -> penalty 9) plus a 160*(1-z)
correction -> 169 total, matching dists=-1 in the reference.

This walrus build rejects instructions carrying >1 sync wait; a post-pass
moves extra waits onto same-engine NoOps.
"""
import sys

sys.path.insert(0, "/opt/trn_rl_repo")

import numpy as np

import bass_rust
import concourse.bass as bass
import concourse.mybir as mybir
from concourse.tile import TileContext
from concourse import bass_utils
from concourse.bass_utils import run_bass_kernel_spmd

B, N, H, W = 512, 8192, 200, 200
NCORES = 8
BPC = B // NCORES            # 64 batches per core
GPB = 8                      # batches per round
ROUNDS = BPC // GPB          # 8
SW = N // 16                 # 512 point-slots per partition per batch
M = 10                       # row-decimation ways
RPT = H // M                 # rows per table (40)
F = RPT * W                  # 8000 elements per table
GK = 512                     # indices per indirect_copy call
HP = 2                       # sigma-buffer chunks per round
HN = N // HP                 # 4096 points per chunk
NSLOT = 2 * ROUNDS

F32 = mybir.dt.float32
U16 = mybir.dt.uint16
AF = mybir.ActivationFunctionType
OP = mybir.AluOpType

PROFILE = False
LAST_EXEC_NS = None

bass_utils.upload_artifacts = lambda tmpdir: "file://" + str(tmpdir)


def _split_multiwaits(nc):
    for fn in nc.m.functions:
        for bb in fn.blocks:
            insts = bb.instructions
            out = []
            changed = False
            for inst in insts:
                si = inst.sync_info
                waits = list(si.on_wait) if si is not None else []
                if len(waits) > 1:
                    for k, w in enumerate(waits[:-1]):
                        nop = bass_rust.InstNoOp(
                            name=f"{inst.name}-w{k}", engine=inst.engine)
                        nop.sync_info = bass_rust.SyncInfo(
                            on_wait=[w], on_update=[])
                        out.append(nop)
                    inst.sync_info = bass_rust.SyncInfo(
                        on_wait=[waits[-1]], on_update=list(si.on_update))
                    changed = True
                out.append(inst)
            if changed:
                bb.instructions = out


def _build():
    nc = bass.Bass()
    for v in (3.0,):
        t = nc.alloc_sbuf_tensor(f"const-float32-{v}", [128, 1], F32)
        nc.gpsimd.memset(t.ap(), v)
        nc.const_aps.aps[(F32, v)] = t.ap()
    nc.all_engine_barrier()
    ops_in = nc.dram_tensor("opState", [BPC, N * 2], F32, kind="ExternalInput")
    env_in = nc.dram_tensor("envs", [BPC, H * W], F32, kind="ExternalInput")
    part_out = nc.dram_tensor("partials", [128, NSLOT], F32,
                              kind="ExternalOutput")

    OS_t = nc.alloc_sbuf_tensor("osig", [128, N, 2], F32)
    G1P = [nc.alloc_sbuf_tensor(f"g1p{m}", [128, SW, 2], F32).ap()
           for m in range(M)]
    G2P = nc.alloc_sbuf_tensor("g2p", [128, SW, 2], F32).ap()

    with TileContext(nc) as tc:
        with (
            tc.tile_pool(name="tbl", bufs=1) as tblp,
            tc.tile_pool(name="pts", bufs=2) as ptsp,
            tc.tile_pool(name="prep", bufs=1) as prp,
            tc.tile_pool(name="widx", bufs=2) as wip,
            tc.tile_pool(name="gout", bufs=1) as gop,
            tc.tile_pool(name="comb", bufs=1) as cbp,
            tc.tile_pool(name="acc", bufs=1) as accp,
        ):
            table = tblp.tile([128, F], F32)
            t3d = table[:].rearrange("p (n d) -> p n d", d=2)  # [128,4000,2]
            partials = accp.tile([128, NSLOT], F32)

            env_ap = env_in.ap()
            ops_ap = ops_in.ap()

            for r in range(ROUNDS):
                b0 = r * GPB
                src = env_ap[b0:b0 + GPB, :].rearrange(
                    "g (h c) -> g h c", c=W)
                for m in range(M):
                    nc.sync.dma_start(out=table[m:128:16, :],
                                      in_=src[:, m:H:M, :])

                P = ptsp.tile([128, 2 * SW], F32)
                nc.sync.dma_start(
                    out=P[:],
                    in_=ops_ap[b0:b0 + GPB, :].rearrange(
                        "g (p f) -> (g p) f", p=16))

                # ---- prep: T = clip(x*10,-99,99)+99 = scaled coord - 0.5
                T = prp.tile([128, 2 * SW], F32)
                nc.vector.tensor_scalar(out=T[:], in0=P[:], scalar1=10.0,
                                        scalar2=-99.0, op0=OP.mult, op1=OP.max)
                nc.vector.tensor_scalar(out=T[:], in0=T[:], scalar1=99.0,
                                        scalar2=99.0, op0=OP.min, op1=OP.add)
                IC = prp.tile([128, 2 * SW], U16)    # floor of scaled coord
                nc.scalar.activation(out=IC[:], in_=T[:], func=AF.Copy)
                ITF = prp.tile([128, 2 * SW], F32)
                nc.scalar.activation(out=ITF[:], in_=IC[:], func=AF.Copy)
                ITFv = ITF[:].rearrange("p (s t) -> p s t", t=2)
                ITFX, ITFY = ITFv[:, :, 0], ITFv[:, :, 1]
                DXF = prp.tile([128, 2 * SW], F32)   # frac - 0.5
                nc.vector.tensor_tensor(out=DXF[:], in0=T[:], in1=ITF[:],
                                        op=OP.subtract)
                DXFv = DXF[:].rearrange("p (s t) -> p s t", t=2)
                DXFX, DXFY = DXFv[:, :, 0], DXFv[:, :, 1]

                # h = ix // M via round(ix/M - (M-1)/(2M)); q = ix - M*h
                HC = prp.tile([128, SW], U16)
                nc.scalar.activation(out=HC[:], in_=ITFX, func=AF.Copy,
                                     scale=1.0 / M, bias=-(M - 1) / (2.0 * M))
                HF = prp.tile([128, SW], F32)
                nc.scalar.activation(out=HF[:], in_=HC[:], func=AF.Copy)
                Q = prp.tile([128, SW], F32)
                nc.vector.tensor_scalar(out=Q[:], in0=HF[:],
                                        scalar1=-float(M), scalar2=None,
                                        op0=OP.mult)
                nc.vector.tensor_tensor(out=Q[:], in0=Q[:], in1=ITFX,
                                        op=OP.add)
                KF = prp.tile([128, SW], F32)        # k = h*W + iy
                nc.vector.tensor_scalar(out=KF[:], in0=HF[:],
                                        scalar1=float(W), scalar2=None,
                                        op0=OP.mult)
                nc.vector.tensor_tensor(out=KF[:], in0=KF[:], in1=ITFY,
                                        op=OP.add)
                IA = wip.tile([128, SW], U16)
                nc.scalar.activation(out=IA[:], in_=KF[:], func=AF.Copy)
                IB = wip.tile([128, SW], U16)
                nc.vector.tensor_scalar(out=IB[:], in0=KF[:],
                                        scalar1=float(W),
                                        scalar2=float(F - 2), op0=OP.add,
                                        op1=OP.min)

                # in-range mask z
                AB = prp.tile([128, 2 * SW], F32)
                nc.scalar.activation(out=AB[:], in_=P[:], func=AF.Abs)
                ABv = AB[:].rearrange("p (s t) -> p s t", t=2)
                ZM = prp.tile([128, SW], F32)
                nc.vector.tensor_tensor(out=ZM[:], in0=ABv[:, :, 0],
                                        in1=ABv[:, :, 1], op=OP.max)
                Z = prp.tile([128, SW], F32)
                nc.vector.tensor_scalar(out=Z[:], in0=ZM[:], scalar1=9.9,
                                        scalar2=None, op0=OP.is_le)
                CD = prp.tile([128, SW], F32)
                nc.scalar.activation(
                    out=CD[:], in_=Z[:], func=AF.Copy, scale=-160.0,
                    bias=160.0,
                    accum_out=partials[:, ROUNDS + r:ROUNDS + r + 1])

                # x/y fracs and row weights ax0 = (1-dx)z, ax1 = dx*z
                DXE = prp.tile([128, SW], F32)
                nc.scalar.activation(out=DXE[:], in_=DXFX, func=AF.Copy,
                                     bias=0.5)
                DYE = prp.tile([128, SW], F32)
                nc.scalar.activation(out=DYE[:], in_=DXFY, func=AF.Copy,
                                     bias=0.5)
                DYC = prp.tile([128, SW], F32)
                nc.scalar.activation(out=DYC[:], in_=DXFY, func=AF.Copy,
                                     scale=-1.0, bias=0.5)
                AX1 = prp.tile([128, SW], F32)
                nc.vector.tensor_tensor(out=AX1[:], in0=DXE[:], in1=Z[:],
                                        op=OP.mult)
                AX0 = prp.tile([128, SW], F32)
                nc.vector.tensor_tensor(out=AX0[:], in0=Z[:], in1=AX1[:],
                                        op=OP.subtract)
                # y-weight planes: A0E = ax0*(1-dy), A0O = ax0*dy, ...
                A0E = prp.tile([128, SW], F32)
                nc.vector.tensor_tensor(out=A0E[:], in0=AX0[:], in1=DYC[:],
                                        op=OP.mult)
                A0O = prp.tile([128, SW], F32)
                nc.vector.tensor_tensor(out=A0O[:], in0=AX0[:], in1=DYE[:],
                                        op=OP.mult)
                A1E = prp.tile([128, SW], F32)
                nc.vector.tensor_tensor(out=A1E[:], in0=AX1[:], in1=DYC[:],
                                        op=OP.mult)
                A1O = prp.tile([128, SW], F32)
                nc.vector.tensor_tensor(out=A1O[:], in0=AX1[:], in1=DYE[:],
                                        op=OP.mult)

                # ---- gather + redistribute (half-round sigma buffers)
                O1 = OS_t.ap()
                for c in range(N // GK):
                    i0, i1 = c * (GK // 16), (c + 1) * (GK // 16)
                    nc.gpsimd.indirect_copy(
                        O1[:, c * GK:(c + 1) * GK, :], t3d,
                        IA[:, i0:i1], True)
                for m in range(M):
                    for qq in range(16):
                        # Alternate the two HWDGE rings (SP + ACT) so the
                        # descriptor storm drains through both queue sets.
                        eng = nc.sync if qq % 2 == 0 else nc.scalar
                        eng.dma_start(
                            out=G1P[m][qq:qq + 113:16, :, :],
                            in_=O1[m:m + 113:16, :, :].rearrange(
                                "g (s u) c -> g s (u c)",
                                u=16)[:, :, 2 * qq:2 * qq + 2])
                O2 = OS_t.ap()
                for c in range(N // GK):
                    i0, i1 = c * (GK // 16), (c + 1) * (GK // 16)
                    nc.gpsimd.indirect_copy(
                        O2[:, c * GK:(c + 1) * GK, :], t3d,
                        IB[:, i0:i1], True)
                for qq in range(16):
                    eng = nc.sync if qq % 2 == 0 else nc.scalar
                    eng.dma_start(
                        out=G2P[qq:qq + 113:16, :, :],
                        in_=O2[0:113:16, :, :].rearrange(
                            "g (s u) c -> g s (u c)",
                            u=16)[:, :, 2 * qq:2 * qq + 2])

                # ---- select rows by q and combine
                # R0* = sum_m [q==m]*G1*[m]            (row ix planes)
                # R1* = sum_{m<M-1} [q==m]*G1*[m+1] + [q==M-1]*G2*
                R0E = cbp.tile([128, SW], F32, tag="r0e")
                R0O = cbp.tile([128, SW], F32, tag="r0o")
                R1E = cbp.tile([128, SW], F32, tag="r1e")
                R1O = cbp.tile([128, SW], F32, tag="r1o")
                TMP = cbp.tile([128, SW], F32, tag="tmp")
                for m in range(M):
                    EQ = cbp.tile([128, SW], F32, tag="eq")
                    nc.vector.tensor_scalar(out=EQ[:], in0=Q[:],
                                            scalar1=float(m), scalar2=None,
                                            op0=OP.is_equal)
                    se = (G1P[m + 1] if m < M - 1 else G2P)[:, :, 0]
                    so = (G1P[m + 1] if m < M - 1 else G2P)[:, :, 1]
                    for dst, g in ((R0E, G1P[m][:, :, 0]),
                                   (R0O, G1P[m][:, :, 1]),
                                   (R1E, se), (R1O, so)):
                        if m == 0:
                            nc.vector.tensor_tensor(out=dst[:], in0=EQ[:],
                                                    in1=g, op=OP.mult)
                        else:
                            nc.vector.tensor_tensor(out=TMP[:], in0=EQ[:],
                                                    in1=g, op=OP.mult)
                            nc.vector.tensor_tensor(out=dst[:], in0=dst[:],
                                                    in1=TMP[:], op=OP.add)

                U = cbp.tile([128, SW], F32, tag="u")
                nc.vector.tensor_tensor(out=U[:], in0=R0E[:], in1=A0E[:],
                                        op=OP.mult)
                UT = cbp.tile([128, SW], F32, tag="ut")
                for g, a in ((R0O, A0O), (R1E, A1E), (R1O, A1O)):
                    nc.vector.tensor_tensor(out=UT[:], in0=g[:], in1=a[:],
                                            op=OP.mult)
                    nc.vector.tensor_tensor(out=U[:], in0=U[:], in1=UT[:],
                                            op=OP.add)
                PS = U
                RT = cbp.tile([128, SW], F32, tag="rt")
                nc.scalar.activation(out=RT[:], in_=PS[:], func=AF.Relu,
                                     scale=-10.0, bias=3.0)
                SQ = cbp.tile([128, SW], F32, tag="sq")
                nc.scalar.activation(out=SQ[:], in_=RT[:], func=AF.Square,
                                     accum_out=partials[:, r:r + 1])

            nc.sync.dma_start(out=part_out.ap()[:], in_=partials[:])

    _split_multiwaits(nc)
    return nc


_CACHE = {}


def kernel(opState, envs):
    global LAST_EXEC_NS
    if "nc" not in _CACHE:
        _CACHE["nc"] = _build()
    nc = _CACHE["nc"]

    opState = np.ascontiguousarray(opState, dtype=np.float32)
    envs = np.ascontiguousarray(envs, dtype=np.float32)
    envs2 = envs.reshape(B, H * W)
    ops2 = opState.reshape(B, N * 2)

    in_maps = []
    for c in range(NCORES):
        sl = slice(c * BPC, (c + 1) * BPC)
        in_maps.append({
            "opState": np.ascontiguousarray(ops2[sl]),
            "envs": np.ascontiguousarray(envs2[sl]),
        })

    res = run_bass_kernel_spmd(nc, in_maps, core_ids=list(range(NCORES)),
                               trace=PROFILE)
    if res.exec_time_ns is not None:
        LAST_EXEC_NS = res.exec_time_ns

    total = np.float64(0.0)
    for r in res.results:
        total += r["partials"].astype(np.float64).sum()
    return np.float32(total / (B * N))


if __name__ == "__main__":
    d = np.load("/root/problem/work/ref_cache.npz")
    out = kernel(d["opState"], d["envs"])
    exp = float(d["out"])
    print("actual:", float(out), "expected:", exp,
          "rel:", abs(float(out) - exp) / abs(exp))

